# revision 16
# baseline (speedup 1.0000x reference)
"""MoE (63 routed experts, top-7, 1 shared expert) Trainium2 Bass kernel.

Strategy (expert parallelism, per sharding hint):
  - Host: router matmul + softmax + top-k (tiny: 0.7 GFLOP vs 220 GFLOP of
    expert FFNs), token gather per expert.
  - Device (8 NeuronCores, SPMD): each core runs 9 "units": 8 routed-expert
    slots (64 slots globally = 63 experts + 1 overflow slot) and 1
    shared-expert slot over a 1/8 token slice.
    Each unit: h = gelu(XeT^T @ W1 + b1); y = h @ W2, with GELU fused into
    the PSUM eviction on the scalar engine and the PSUM->SBUF output
    eviction on the vector engine.  Weights are host-pretiled into
    [128, KO, I] layout so each weight tensor loads in two flat DMAs.
  - Routed slots run in fp8 e4m3 with DoubleRow matmuls (2 contraction
    k-tiles per instruction, 2x PE rate).  Routed-expert output errors are
    attenuated by their gates (sum gates^2 ~ 0.015), so fp8 noise there is
    cheap; the shared expert (gate 1) dominates the error budget and stays
    in fp16 at full rate.  Weights are pre-scaled by 32 on the host so fp8
    encodings sit in the normal range; the GELU eviction descales via the
    activation `scale` input and the host descales the second matmul
    during the gather (folded into the gates).
  - DMA descriptor generation is spread across engine queues (weights on
    the idle Pool/GpSimd queue, activations + output drains on SP/Sync) so
    unit-boundary descriptor bursts don't starve the PE; weight/activation
    tiles for unit u+1 prefetch during unit u via pool double-buffering.
  - Host: scatter-add gated expert outputs (+ gate*b2), add shared out,
    bias and residual.

Experts are assigned to slots by descending load rank with static per-unit
token capacities (CAPS); both matmul layers' free dim is the capacity, so
PE cost tracks actual expert load.  Overload spills into the spare 64th
slot and, beyond that, to an exact host-side FFN for the few excess
tokens.  Gating and b2 are applied on the host during the scatter.
"""

import os

import numpy as np

B, S, HID = 2, 2048, 1280
E = 63
I = 1280
TOP_K = 7
NCORES = 8
UNITS = 9          # 8 expert slots + 1 shared-expert slot
C = 512            # token capacity per expert slot
KO = HID // 128    # 10 contraction chunks
T = B * S          # 4096
TSH = T // NCORES  # 512 shared-expert tokens per core

WHEAD = 128        # weight columns in the head DMA (unblocks first matmuls)

# Per-unit-index token capacities. Experts are assigned to slots by load
# rank (rank r -> core r%8, unit r//8), so unit j only ever sees the j-th
# bucket of the descending load distribution; caps cover the bucket maxima
# of any near-uniform routing with margin. Uncovered overflow goes to the
# spare slot 63 and, beyond that, to an exact host fallback.
CAPS = [512, 492, 476, 464, 452, 440, 432, 424, C]   # unit 8 = shared

WSCALE = 32.0      # fp8 weight pre-scale (keeps encodings in normal range)

# "fp8": routed slots fp8 e4m3 DoubleRow (2x PE), shared slot fp16.
# "fp16": everything fp16 (fallback).
WORK_DTYPE = os.environ.get("MOE_WDT", "fp8")

_cache = {}


def _build_nc(wdt):
    import concourse.mybir as mybir
    import concourse.tile as tile
    from concourse import bacc

    f32 = mybir.dt.float32
    f16 = mybir.dt.float16
    f8 = mybir.dt.float8e4
    GELU = mybir.ActivationFunctionType.Gelu
    DR = mybir.MatmulPerfMode.DoubleRow

    rdt = f8 if wdt == "fp8" else f16   # routed-slot dtype

    nc = bacc.Bacc(None, target_bir_lowering=False)

    # routed slots (units 0-7); weights pretiled [128, KO, cols]
    xg_d = nc.dram_tensor("xg", [8, 128, KO, C], rdt, kind="ExternalInput")
    w1_d = nc.dram_tensor("w1", [8, 128, KO, I], rdt, kind="ExternalInput")
    w2_d = nc.dram_tensor("w2", [8, 128, KO, HID], rdt, kind="ExternalInput")
    # all-unit biases in one tensor: b1a[p, u, ko]
    b1_d = nc.dram_tensor("b1", [128, UNITS, KO], f32, kind="ExternalInput")
    # shared-expert slot (unit 8), fp16
    xs_d = nc.dram_tensor("xs", [128, KO, C], f16, kind="ExternalInput")
    w1s_d = nc.dram_tensor("w1s", [128, KO, I], f16, kind="ExternalInput")
    w2s_d = nc.dram_tensor("w2s", [128, KO, HID], f16, kind="ExternalInput")
    # transposed output: out[u, p, hk, c] = y[token c, h = hk*128+p]
    out_d = nc.dram_tensor("out", [UNITS, 128, KO, C], f16,
                           kind="ExternalOutput")

    with tile.TileContext(nc) as tc:
        with tc.tile_pool(name="xg_p", bufs=3) as xg_p, \
             tc.tile_pool(name="h1_p", bufs=2) as h1_p, \
             tc.tile_pool(name="wh_p", bufs=2) as wh_p, \
             tc.tile_pool(name="wr_p", bufs=2) as wr_p, \
             tc.tile_pool(name="w2_p", bufs=2) as w2_p, \
             tc.tile_pool(name="out_p", bufs=2) as out_p, \
             tc.tile_pool(name="sm_p", bufs=1) as sm_p, \
             tc.tile_pool(name="ps1_p", bufs=3, space="PSUM") as ps1_p, \
             tc.tile_pool(name="ps2_p", bufs=4, space="PSUM") as ps2_p:

            b1a = sm_p.tile([128, UNITS, KO], f32, tag="b1a")
            nc.scalar.dma_start(b1a[:], b1_d[:])

            for u in range(UNITS):
                CAP = CAPS[u]
                shared = u == 8
                udt = f16 if shared else rdt
                dr = (udt == f8)

                # weights in separate head/rest tiles: hazard tracking is
                # per-tile on DMA writes, so the first matmuls must only
                # wait for the small head transfer, not the whole weight.
                # Descriptors go on the otherwise-idle Pool queue.
                w1src = w1s_d if shared else w1_d[u]
                w1h = wh_p.tile([128, KO, WHEAD], udt, tag="w1h", name="w1h")
                nc.gpsimd.dma_start(w1h[:], w1src[:, :, :WHEAD])
                w1r = wr_p.tile([128, KO, I - WHEAD], udt, tag="w1r",
                                name="w1r")
                nc.gpsimd.dma_start(w1r[:], w1src[:, :, WHEAD:])

                # x slices likewise split so the first k-pair unblocks early
                xsrc = xs_d if shared else xg_d[u]
                xua = xg_p.tile([128, 2, C], udt, tag="xua")
                nc.sync.dma_start(xua[:, :, :CAP], xsrc[:, :2, :CAP])
                xub = xg_p.tile([128, KO - 2, C], udt, tag="xub")
                nc.sync.dma_start(xub[:, :, :CAP], xsrc[:, 2:, :CAP])

                w2t = w2_p.tile([128, KO, HID], udt, tag="w2t", name="w2t")
                w2src = w2s_d if shared else w2_d[u]
                nc.gpsimd.dma_start(w2t[:], w2src[:])

                def xk(ko):
                    # x k-tile ko as [128, CAP] view
                    return (xua[:, ko, :CAP] if ko < 2
                            else xub[:, ko - 2, :CAP])

                def xk2(kp):
                    # x k-pair 2kp..2kp+1 as [128, 2, CAP] view
                    return (xua[:, :, :CAP] if kp == 0
                            else xub[:, 2 * kp - 2:2 * kp, :CAP])

                def w1v(ko_lo, ko_n, io):
                    # w1 [128, ko_n, 128] view for output group io
                    j = io * 128
                    if j < WHEAD:
                        return w1h[:, ko_lo:ko_lo + ko_n, j:j + 128]
                    return w1r[:, ko_lo:ko_lo + ko_n,
                               j - WHEAD:j - WHEAD + 128]

                def w1v1(ko, io):
                    # w1 [128, 128] view (single k-tile) for output group io
                    j = io * 128
                    if j < WHEAD:
                        return w1h[:, ko, j:j + 128]
                    return w1r[:, ko, j - WHEAD:j - WHEAD + 128]

                h1 = h1_p.tile([128, KO, C], udt, tag="h1")

                # ---- mm1: h1[i, c] = gelu(sum_h W1[h,i] * X^T[h,c] + b1[i])
                for io in range(KO):
                    ps = ps1_p.tile([128, C], f32, tag="ps1")
                    if dr:
                        for kp in range(KO // 2):
                            nc.tensor.matmul(
                                ps[:, :CAP],
                                w1v(2 * kp, 2, io),
                                xk2(kp),
                                start=(kp == 0),
                                stop=(kp == KO // 2 - 1),
                                perf_mode=DR,
                            )
                    else:
                        for ko in range(KO):
                            nc.tensor.matmul(
                                ps[:, :CAP],
                                w1v1(ko, io),
                                xk(ko),
                                start=(ko == 0),
                                stop=(ko == KO - 1),
                            )
                    nc.scalar.activation(
                        h1[:, io, :CAP], ps[:, :CAP], GELU,
                        bias=b1a[:, u, io:io + 1],
                        scale=(1.0 / WSCALE) if dr else 1.0)

                # ---- mm2 (transposed): yT[h, c] = sum_i W2[i, h] * h1[i, c]
                # gating, b2 and the fp8 descale are applied on the host.
                oy = out_p.tile([128, KO, C], f16, tag="oy")
                for hk in range(KO):
                    ps2 = ps2_p.tile([128, C], f32, tag="ps2")
                    if dr:
                        for kp in range(KO // 2):
                            nc.tensor.matmul(
                                ps2[:, :CAP],
                                w2t[:, 2 * kp:2 * kp + 2,
                                    hk * 128:(hk + 1) * 128],
                                h1[:, 2 * kp:2 * kp + 2, :CAP],
                                start=(kp == 0),
                                stop=(kp == KO // 2 - 1),
                                perf_mode=DR,
                            )
                    else:
                        for ko in range(KO):
                            nc.tensor.matmul(
                                ps2[:, :CAP],
                                w2t[:, ko, hk * 128:(hk + 1) * 128],
                                h1[:, ko, :CAP],
                                start=(ko == 0),
                                stop=(ko == KO - 1),
                            )
                    nc.vector.tensor_copy(oy[:, hk, :CAP], ps2[:, :CAP])
                    # drain rows 0-7 mid-unit, 8-9 at the end so the final
                    # DMA (and the kernel tail) stays small; the last unit
                    # drains in three pieces so its final DMA is tiny
                    if u < UNITS - 1:
                        if hk == 7:
                            nc.sync.dma_start(out_d[u, :, :8, :CAP],
                                              oy[:, :8, :CAP])
                        elif hk == 9:
                            nc.sync.dma_start(out_d[u, :, 8:, :CAP],
                                              oy[:, 8:, :CAP])
                    else:
                        # idle Pool queue/ring at the end of the run
                        if hk == 5:
                            nc.gpsimd.dma_start(out_d[u, :, :6, :CAP],
                                                oy[:, :6, :CAP])
                        elif hk == 8:
                            nc.gpsimd.dma_start(out_d[u, :, 6:9, :CAP],
                                                oy[:, 6:9, :CAP])
                        elif hk == 9:
                            nc.gpsimd.dma_start(out_d[u, :, 9:, :CAP],
                                                oy[:, 9:, :CAP])

    nc.compile()
    return nc


def _get_nc(wdt):
    if wdt not in _cache:
        _cache[wdt] = _build_nc(wdt)
    return _cache[wdt]


def _np_rdt(wdt):
    if wdt == "fp8":
        import ml_dtypes
        if os.environ.get("MOE_FP8ENC", "ieee") == "fn":
            return np.dtype(ml_dtypes.float8_e4m3fn)
        return np.dtype(ml_dtypes.float8_e4m3)
    return np.dtype(np.float16)


def _gelu_np(v):
    from scipy.special import erf
    v = v.astype(np.float32)
    return (0.5 * v * (1.0 + erf(v / np.sqrt(2.0)))).astype(np.float32)


def _tile_w(w):
    # [K, N] -> [128, KO, N] with wt[p, ko, n] = w[ko*128+p, n]
    return w.reshape(KO, 128, w.shape[1]).transpose(1, 0, 2)


def _ensure_axon_hooks_stub():
    """bass_utils' axon trace path imports antenv.axon_hooks, which this
    image lacks; provide a no-op stub so a BASS_TRACE-enabled environment
    degrades gracefully instead of crashing."""
    import sys
    import types
    try:
        import antenv.axon_hooks  # noqa: F401
        return
    except ImportError:
        pass
    try:
        import antenv
    except ImportError:
        return
    mod = types.ModuleType("antenv.axon_hooks")
    holder = [None]
    mod.set_axon_ntff_profile_hook = lambda h: holder.__setitem__(0, h)
    mod.get_axon_ntff_profile_hook = lambda: holder[0]
    sys.modules["antenv.axon_hooks"] = mod
    antenv.axon_hooks = mod


def kernel(x, w1_shared, b1_shared, w2_shared, b2_shared,
           router_w, router_b, w1, b1, w2, b2):
    _ensure_axon_hooks_stub()
    from concourse.bass_utils import run_bass_kernel_spmd

    wdt = WORK_DTYPE
    fp8 = wdt == "fp8"
    rdt = _np_rdt(wdt)
    ws = WSCALE if fp8 else 1.0

    x = np.asarray(x, np.float32)
    w1 = np.asarray(w1, np.float32)
    b1 = np.asarray(b1, np.float32)
    w2 = np.asarray(w2, np.float32)
    b2 = np.asarray(b2, np.float32)
    w1_shared = np.asarray(w1_shared, np.float32)
    b1_shared = np.asarray(b1_shared, np.float32)
    w2_shared = np.asarray(w2_shared, np.float32)
    b2_shared = np.asarray(b2_shared, np.float32)
    router_w = np.asarray(router_w, np.float32)
    router_b = np.asarray(router_b, np.float32)

    xf = x.reshape(T, HID)

    # ---------------- host routing ----------------
    logits = xf @ router_w + router_b
    m = logits.max(-1, keepdims=True)
    ex = np.exp(logits - m, dtype=np.float32)
    affin = ex / ex.sum(-1, keepdims=True, dtype=np.float32)
    order = np.argsort(-affin, axis=-1, kind="stable")[:, :TOP_K]   # [T, K]
    vals = np.take_along_axis(affin, order, axis=-1)                # [T, K]

    # group (token, gate) pairs by expert
    flat_e = order.ravel()
    flat_t = np.repeat(np.arange(T), TOP_K)
    flat_g = vals.ravel()
    sort = np.argsort(flat_e, kind="stable")
    se, st, sg = flat_e[sort], flat_t[sort], flat_g[sort]
    starts = np.searchsorted(se, np.arange(E + 1))
    tok_by_e = [st[starts[e]:starts[e + 1]] for e in range(E)]
    gate_by_e = [sg[starts[e]:starts[e + 1]] for e in range(E)]

    # slot table: 64 expert slots; slot s = core*8 + unit.  Experts are
    # assigned by descending load rank: rank r -> core r%8, unit r//8, so
    # every core gets one expert from each load bucket and unit j's static
    # capacity CAPS[j] covers its bucket maximum.
    NSLOT = NCORES * 8
    slot_expert = [-1] * NSLOT
    slot_tok = [np.empty(0, np.int64)] * NSLOT
    slot_gate = [np.empty(0, np.float32)] * NSLOT
    ranked = sorted(range(E), key=lambda e: -len(tok_by_e[e]))
    overflow = []   # (expert, tokens, gates) beyond the primary slot cap
    for r, e in enumerate(ranked):
        s = (r % NCORES) * 8 + (r // NCORES)
        cap = CAPS[r // NCORES]
        slot_expert[s] = e
        slot_tok[s] = tok_by_e[e][:cap]
        slot_gate[s] = gate_by_e[e][:cap]
        if len(tok_by_e[e]) > cap:
            overflow.append((e, tok_by_e[e][cap:], gate_by_e[e][cap:]))
    # worst overflow spills into the spare slot 63 (unit 7, cap CAPS[7]);
    # anything further goes to an exact host fallback (rare).
    host_fallback = []
    if overflow:
        overflow.sort(key=lambda t: -len(t[1]))
        e0, t0, g0 = overflow[0]
        cap63 = CAPS[7]
        slot_expert[63] = e0
        slot_tok[63] = t0[:cap63]
        slot_gate[63] = g0[:cap63]
        if len(t0) > cap63:
            host_fallback.append((e0, t0[cap63:], g0[cap63:]))
        for e, t, g in overflow[1:]:
            host_fallback.append((e, t, g))

    # ---------------- build per-core device inputs ----------------
    # x transposed + partition-tiled: xT_t[ko, p, t] = x[t, ko*128+p]
    xT32 = np.ascontiguousarray(xf.T).reshape(KO, 128, T)
    xT_r = xT32.astype(rdt)                 # routed slots (fp8 or fp16)
    xT_s = xT32.astype(np.float16)          # shared slot

    w1t_sh = _tile_w(w1_shared[0]).astype(np.float16)
    w2t_sh = _tile_w(w2_shared[0]).astype(np.float16)
    b1t_sh = np.ascontiguousarray(b1_shared[0].reshape(KO, 128).T)

    in_maps = []
    for c in range(NCORES):
        xg = np.zeros((8, 128, KO, C), rdt)
        w1u = np.zeros((8, 128, KO, I), rdt)
        b1a = np.zeros((128, UNITS, KO), np.float32)
        w2u = np.zeros((8, 128, KO, HID), rdt)
        for u in range(8):
            s = c * 8 + u
            e = slot_expert[s]
            if e < 0 or len(slot_tok[s]) == 0:
                continue
            n = len(slot_tok[s])
            idx = np.zeros(C, np.int64)
            idx[:n] = slot_tok[s]
            xg[u] = xT_r[:, :, idx].swapaxes(0, 1)
            w1u[u] = _tile_w(ws * w1[e]).astype(rdt)
            b1a[:, u, :] = b1[e].reshape(KO, 128).T
            w2u[u] = _tile_w(ws * w2[e]).astype(rdt)
        b1a[:, 8, :] = b1t_sh
        # shared-expert unit
        xs = np.ascontiguousarray(
            xT_s[:, :, c * TSH:(c + 1) * TSH].swapaxes(0, 1))
        in_maps.append({"xg": xg, "w1": w1u, "b1": b1a, "w2": w2u,
                        "xs": xs, "w1s": w1t_sh, "w2s": w2t_sh})

    # ---------------- run on 8 cores ----------------
    nc = _get_nc(wdt)
    res = run_bass_kernel_spmd(nc, in_maps, core_ids=list(range(NCORES)))
    outs = [r["out"] for r in res.results]   # [UNITS, 128, KO, C] f16 each

    # ---------------- host unshard / scatter ----------------
    # device output is transposed: outs[c][u][p, hk, c'] = y[c', hk*128+p]
    def untile_y(o, n):
        return o.transpose(1, 0, 2).reshape(HID, C)[:, :n].T.astype(np.float32)

    acc = np.zeros((T, HID), np.float32)     # shared + routed
    # shared expert (unit 8 on each core), gate 1, + b2_shared
    for c in range(NCORES):
        ys = untile_y(outs[c][8], TSH)
        acc[c * TSH:(c + 1) * TSH] = ys + b2_shared[0]
    # routed experts: gate * (y/ws + b2), scattered by token
    for s in range(NCORES * 8):
        e = slot_expert[s]
        n = len(slot_tok[s])
        if e < 0 or n == 0:
            continue
        ye = untile_y(outs[s // 8][s % 8], n)
        g = slot_gate[s][:, None]
        # token indices are unique within one slot, so fancy += is safe
        acc[slot_tok[s]] += (g / ws) * ye + g * b2[e][None, :]
    # exact host fallback for overflow beyond device capacity
    for e, toks, gs in host_fallback:
        h = _gelu_np(xf[toks] @ w1[e] + b1[e])
        acc[toks] += gs[:, None] * (h @ w2[e] + b2[e])

    return (acc + xf).reshape(B, S, HID).astype(np.float32)
